# revision 29
# baseline (speedup 1.0000x reference)
"""Trainium2 Bass kernel for nn_DiffusionLayer (gnn_message_passing).

Computation (full shapes):
  x (16,64,64,512), A (16,512,64,64), phys_prior (16,64,512) ->
  corr (16,32,64,512)

Sharding: pure data parallel over batch B=16 across 8 cores (B_LOC=2 each).
All reductions are local to a (b, m) tile; scalar params replicated.

Per-core layout convention: SBUF partition index p = b_loc*64 + c  (128
partitions), free dim = m (512).  This makes every elementwise stage a
[128, 512] tile and every DMA partition-major with >=256B runs.

Stages:
  1. s = mean_f x           : PE matmul with a block-diagonal ones lhsT
                              (K=(f2,c)=128, N=512), accumulated in PSUM.
  2. deg^T[(b,c), m]        : DVE reduce over innermost d of A tiles
                              [128=(b,c), (m_T, d)].
  3. As^T[(b,c), m]         : 2 small PE matmuls per m (per b): out[d,1] =
                              A[b,m](c,d)^T @ s[b,:,m], written directly at
                              psum partitions b*64..b*64+64 (PE quadrant
                              tiling), one psum bank holds all of As^T.
  4. tiny MLP r(b,c)        : DVE/ACT ops on [128, 16] tiles.
  5. combine                : s_new = s*(1-DT*k*deg) + DT*k*As + DT*alpha*pp
                              + DT*r    (DVE, [128,512] tiles)
  6. out[o] = s_new*pw[o]+pb[o] : DVE tensor_scalar per channel, DMA out.
"""

import os
import sys
import math
import numpy as np

sys.path.insert(0, "/opt/trn_rl_repo")

import concourse.bass as bass  # noqa: E402
from concourse import bacc  # noqa: E402
import concourse.tile as tile  # noqa: E402
from concourse import mybir  # noqa: E402
from concourse.bass_utils import run_bass_kernel_spmd  # noqa: E402

B, F_DIM, C, M = 16, 64, 64, 512
OUT_CH = 32
DT = 1.0
N_CORES = 8
B_LOC = B // N_CORES  # 2
F32 = mybir.dt.float32
M_T = 32  # m's per A tile

_CACHE = {}


def _build_bass():
    nc = bacc.Bacc()

    x_sh = nc.declare_dram_parameter("x_sh", [B_LOC, F_DIM, C, M], F32, isOutput=False)
    a_sh = nc.declare_dram_parameter("a_sh", [B_LOC, M, C, C], F32, isOutput=False)
    pp_sh = nc.declare_dram_parameter("pp_sh", [B_LOC, C, M], F32, isOutput=False)
    ones_bd = nc.declare_dram_parameter("ones_bd", [128, C], F32, isOutput=False)
    w1r = nc.declare_dram_parameter("w1r", [128, 16], F32, isOutput=False)
    b1r = nc.declare_dram_parameter("b1r", [128, 16], F32, isOutput=False)
    w2r = nc.declare_dram_parameter("w2r", [128, 16], F32, isOutput=False)
    cvec = nc.declare_dram_parameter("cvec", [128, 4], F32, isOutput=False)
    pwpb = nc.declare_dram_parameter("pwpb", [128, 2 * OUT_CH], F32, isOutput=False)
    out_sh = nc.declare_dram_parameter("out", [B_LOC, OUT_CH, C, M], F32, isOutput=True)

    AX = mybir.AxisListType
    OP = mybir.AluOpType
    ACTF = mybir.ActivationFunctionType

    with tile.TileContext(nc) as tc:
        with (
            tc.tile_pool(name="const", bufs=1) as cpool,
            tc.tile_pool(name="xp", bufs=2) as xpool,
            tc.tile_pool(name="ap", bufs=14) as apool,
            tc.tile_pool(name="sp", bufs=1) as spool,
            tc.tile_pool(name="tmp", bufs=1) as tpool,
            tc.tile_pool(name="dpk", bufs=4) as dpkpool,
            tc.tile_pool(name="small", bufs=1) as smpool,
            tc.tile_pool(name="op", bufs=2) as opool,
            tc.tile_pool(name="ps_s", bufs=1, space="PSUM") as ps_s_pool,
            tc.tile_pool(name="ps_as", bufs=3, space="PSUM") as ps_as_pool,
        ):
            # ---- constants ----
            ones_t = cpool.tile([128, C], F32)
            nc.sync.dma_start(ones_t[:], ones_bd[:])
            w1r_t = cpool.tile([128, 16], F32)
            nc.sync.dma_start(w1r_t[:], w1r[:])
            b1r_t = cpool.tile([128, 16], F32)
            nc.sync.dma_start(b1r_t[:], b1r[:])
            w2r_t = cpool.tile([128, 16], F32)
            nc.sync.dma_start(w2r_t[:], w2r[:])
            cvec_t = cpool.tile([128, 4], F32)
            nc.sync.dma_start(cvec_t[:], cvec[:])
            pwpb_t = cpool.tile([128, 2 * OUT_CH], F32)
            nc.sync.dma_start(pwpb_t[:], pwpb[:])
            pp_t = spool.tile([128, M], F32)
            nc.sync.dma_start(pp_t[:], pp_sh[:])

            # ---- stage 1: s = mean_f x  (PE blockdiag-ones matmul) ----
            # One DMA covers FPG f-pairs (2 MiB transfers -> all 16 SDMA
            # engines; 2 KiB descriptors).  Free layout (fp, m).
            s_ps = ps_s_pool.tile([128, M], F32)
            s_t = spool.tile([128, M], F32)
            s_bds = []
            NFP = F_DIM // 2  # f-pairs
            FPG = 8  # f-pairs per DMA
            for b in range(B_LOC):
                for fg in range(NFP // FPG):
                    xt = xpool.tile([128, FPG * M], F32)
                    # in: (fp, f2, c, m); out traversal (f2, c | fp, m)
                    xin = x_sh[b, 2 * fg * FPG : 2 * (fg + 1) * FPG].rearrange(
                        "(fp ftwo) c m -> ftwo c fp m", ftwo=2
                    )
                    nc.sync.dma_start(xt[:].rearrange("p (fp m) -> p fp m", m=M), xin)
                    for g in range(FPG):
                        fp = fg * FPG + g
                        nc.tensor.matmul(
                            s_ps[b * C : (b + 1) * C, :],
                            ones_t[:],
                            xt[:, g * M : (g + 1) * M],
                            start=(fp == 0),
                            stop=(fp == NFP - 1),
                        )
                # s and the blockdiag-s for this b become available as soon
                # as this b's x stream finishes -> b's As matmuls can start
                # draining A tiles while the other b's x is still streaming.
                bsl = slice(b * C, (b + 1) * C)
                nc.scalar.activation(
                    s_t[bsl, :], s_ps[bsl, :], ACTF.Copy, scale=1.0 / F_DIM
                )
                bb = spool.tile([128, M], F32, tag=f"sbd{b}", name=f"sbd{b}")
                nc.vector.memset(bb[:], 0.0)
                nc.vector.tensor_copy(bb[0:64, 0::2], s_t[bsl, 0::2])
                nc.vector.tensor_copy(bb[64:128, 1::2], s_t[bsl, 1::2])
                s_bds.append(bb)

            # ---- stage 4 (early): tiny MLP on r_in = mean_m s ----
            rsum = smpool.tile([128, 1], F32)
            nc.vector.tensor_reduce(rsum[:], s_t[:], axis=AX.X, op=OP.add)
            rin = smpool.tile([128, 1], F32)
            nc.vector.tensor_scalar_mul(rin[:], rsum[:], 1.0 / M)
            hp = smpool.tile([128, 16], F32)
            nc.vector.tensor_scalar(hp[:], w1r_t[:], rin[:], None, op0=OP.mult)
            nc.vector.tensor_add(hp[:], hp[:], b1r_t[:])
            hneg = smpool.tile([128, 16], F32)
            nc.vector.tensor_scalar_min(hneg[:], hp[:], 0.0)
            hexp = smpool.tile([128, 16], F32)
            nc.scalar.activation(hexp[:], hneg[:], ACTF.Exp)
            hrelu = smpool.tile([128, 16], F32)
            nc.vector.tensor_scalar_max(hrelu[:], hp[:], 0.0)
            helu = smpool.tile([128, 16], F32)
            nc.vector.tensor_add(helu[:], hexp[:], hrelu[:])
            # helu currently = elu + 1 ; fold the -1 into rdt via dot with w2r:
            # sum(w2r*(elu+1)) = sum(w2r*elu) + sum(w2r)  -> subtract sum(w2r)
            hw = smpool.tile([128, 16], F32)
            nc.vector.tensor_mul(hw[:], helu[:], w2r_t[:])
            rpre = smpool.tile([128, 1], F32)
            nc.vector.tensor_reduce(rpre[:], hw[:], axis=AX.X, op=OP.add)
            # rdt = rpre - sum(w2r) + DT*b2  (host folds both into cvec[:,3])
            rdt = smpool.tile([128, 1], F32)
            nc.vector.tensor_scalar(rdt[:], rpre[:], cvec_t[:, 3:4], None, op0=OP.add)

            # ---- stages 2+3+5+6, software-pipelined in m-quarters ----
            # Emit quarter q's combine+out AFTER quarter q+1's A-loop so the
            # DVE FIFO never blocks the A stream; out-DMAs ride the scalar
            # HWDGE ring so they cannot head-of-line-block A-DMAs (sync ring).
            deg_t = spool.tile([128, M], F32)
            snew = spool.tile([128, M], F32)
            MH = M_T // 2  # m-pairs per tile
            NQ = 4
            MBH = M // NQ  # m's per quarter
            OG = 8  # out channels per DMA

            as_tiles = {}

            def emit_a_quarter(q):
                as_tiles[q] = []
                for b in range(B_LOC):
                    aspb = ps_as_pool.tile(
                        [128, MBH], F32, tag=f"asps{b}", name=f"asps{b}_{q}"
                    )
                    as_tiles[q].append(aspb)
                for mt in range(q * (MBH // M_T), (q + 1) * (MBH // M_T)):
                    for b in range(B_LOC):
                        at = apool.tile([128, MH * C], F32, tag=f"at{b}")
                        ain = a_sh[b, mt * M_T : (mt + 1) * M_T].rearrange(
                            "(m1 m0) c d -> m0 c m1 d", m0=2
                        )
                        nc.sync.dma_start(
                            at[:].rearrange("p (m d) -> p m d", d=C), ain
                        )
                        dpk = dpkpool.tile([128, MH], F32, tag="dpk")
                        at3 = at[:].rearrange("p (mm d) -> p mm d", d=C)
                        nc.vector.tensor_reduce(dpk[:], at3, axis=AX.X, op=OP.add)
                        bsl = slice(b * C, (b + 1) * C)
                        nc.vector.tensor_copy(
                            deg_t[bsl, mt * M_T : (mt + 1) * M_T : 2], dpk[0:64, :]
                        )
                        nc.vector.tensor_copy(
                            deg_t[bsl, mt * M_T + 1 : (mt + 1) * M_T : 2],
                            dpk[64:128, :],
                        )
                        for j in range(MH // 2):
                            # [128,128] weight covers 4 m's (one LDW);
                            # rhs = 4 blockdiag-s cols; out rows (m1p, d),
                            # psum col == m - q*MBH
                            me4 = mt * M_T + 4 * j
                            mq = me4 - q * MBH
                            nc.tensor.matmul(
                                as_tiles[q][b][:, mq : mq + 4],
                                at[:, 2 * j * C : (2 * j + 2) * C],
                                s_bds[b][:, me4 : me4 + 4],
                                start=True,
                                stop=True,
                            )

            def emit_combine_out(q):
                as_ps_b = as_tiles.pop(q)
                hs = slice(q * MBH, (q + 1) * MBH)
                t2p = tpool.tile([128, MBH], F32, tag="t2p")
                nc.vector.tensor_scalar(
                    t2p[:], deg_t[:, hs], cvec_t[:, 0:1], 1.0, op0=OP.mult, op1=OP.add
                )
                t2 = tpool.tile([128, MBH], F32, tag="t2")
                nc.vector.tensor_mul(t2[:], t2p[:], s_t[:, hs])
                # t3 = DT*k*As: psum rows (m1-parity, d); valid half by
                # (m//2)%2: cols {4u,4u+1} -> rows 0:64, {4u+2,4u+3} -> 64:128
                t3 = tpool.tile([128, MBH], F32, tag="t3")
                kap = cvec_t[0:64, 1:2]
                for b in range(B_LOC):
                    bsl = slice(b * C, (b + 1) * C)
                    aps = as_ps_b[b]
                    t3v = t3[bsl, :].rearrange("p (u k) -> p u k", k=4)
                    apse = aps[0:64, :].rearrange("p (u k) -> p u k", k=4)
                    apso = aps[64:128, :].rearrange("p (u k) -> p u k", k=4)
                    nc.vector.tensor_scalar(
                        t3v[:, :, 0:2], apse[:, :, 0:2], kap, None, op0=OP.mult
                    )
                    nc.vector.tensor_scalar(
                        t3v[:, :, 2:4], apso[:, :, 2:4], kap, None, op0=OP.mult
                    )
                t4 = tpool.tile([128, MBH], F32, tag="t4")
                nc.vector.tensor_add(t4[:], t2[:], t3[:])
                t5 = tpool.tile([128, MBH], F32, tag="t5")
                nc.vector.tensor_scalar(
                    t5[:], pp_t[:, hs], cvec_t[:, 2:3], rdt[:], op0=OP.mult, op1=OP.add
                )
                nc.vector.tensor_add(snew[:, hs], t4[:], t5[:])
                for og in range(OUT_CH // OG):
                    ot = opool.tile([128, OG * MBH], F32, tag="ot")
                    for g in range(OG):
                        o = og * OG + g
                        if g % 2 == 0:
                            nc.vector.tensor_scalar(
                                ot[:, g * MBH : (g + 1) * MBH],
                                snew[:, hs],
                                pwpb_t[:, 2 * o : 2 * o + 1],
                                pwpb_t[:, 2 * o + 1 : 2 * o + 2],
                                op0=OP.mult,
                                op1=OP.add,
                            )
                        else:
                            nc.scalar.activation(
                                ot[:, g * MBH : (g + 1) * MBH],
                                snew[:, hs],
                                ACTF.Identity,
                                bias=pwpb_t[:, 2 * o + 1 : 2 * o + 2],
                                scale=pwpb_t[:, 2 * o : 2 * o + 1],
                            )
                    for b in range(B_LOC):
                        odst = out_sh[
                            b, og * OG : (og + 1) * OG, :, q * MBH : (q + 1) * MBH
                        ].rearrange("o c m -> c o m")
                        osrc = ot[b * C : (b + 1) * C, :].rearrange(
                            "p (o m) -> p o m", m=MBH
                        )
                        nc.scalar.dma_start(odst, osrc)

            for q in range(NQ):
                emit_a_quarter(q)
                if q >= 1:
                    emit_combine_out(q - 1)
            emit_combine_out(NQ - 1)

    nc.compile()
    return nc


def _get_bass():
    if "nc" not in _CACHE:
        _CACHE["nc"] = _build_bass()
    return _CACHE["nc"]


def _host_consts(kappa, alpha, w1, b1, w2, b2, pw, pb):
    kappa = float(np.asarray(kappa))
    alpha = float(np.asarray(alpha))
    w1 = np.asarray(w1, np.float32).reshape(16, 1)
    b1 = np.asarray(b1, np.float32).reshape(16)
    w2 = np.asarray(w2, np.float32).reshape(1, 16)
    b2 = np.asarray(b2, np.float32).reshape(1)
    pw = np.asarray(pw, np.float32).reshape(OUT_CH)
    pb = np.asarray(pb, np.float32).reshape(OUT_CH)

    kDT = DT * float(np.log1p(np.exp(kappa)))  # DT * softplus(kappa)

    ones_bd = np.zeros((128, C), np.float32)
    for f in range(2):
        for c in range(C):
            ones_bd[f * C + c, c] = 1.0

    w1r = np.tile(w1.T.astype(np.float32), (128, 1))  # [128,16]
    b1r = np.tile(b1[None, :], (128, 1)).astype(np.float32)
    w2r_dt = np.tile((DT * w2).astype(np.float32), (128, 1))  # [128,16]

    cvec = np.zeros((128, 4), np.float32)
    cvec[:, 0] = -kDT
    cvec[:, 1] = kDT
    cvec[:, 2] = DT * alpha
    # rdt = rpre + cvec3 where rpre = sum(w2r_dt * (elu+1));
    # true DT*r = sum(w2r_dt*elu) + DT*b2  ->  cvec3 = DT*b2 - sum(w2r_dt row)
    cvec[:, 3] = DT * b2[0] - float(w2r_dt[0].sum())

    pwpb = np.zeros((128, 2 * OUT_CH), np.float32)
    pwpb[:, 0::2] = pw[None, :]
    pwpb[:, 1::2] = pb[None, :]
    return ones_bd, w1r, b1r, w2r_dt, cvec, pwpb


def kernel(x, A, phys_prior, kappa, alpha, w1, b1, w2, b2, pw, pb):
    x = np.ascontiguousarray(np.asarray(x, np.float32))
    A = np.ascontiguousarray(np.asarray(A, np.float32))
    phys_prior = np.ascontiguousarray(np.asarray(phys_prior, np.float32))
    ones_bd, w1r, b1r, w2r_dt, cvec, pwpb = _host_consts(
        kappa, alpha, w1, b1, w2, b2, pw, pb
    )

    nc = _get_bass()
    core_ids = list(range(N_CORES))
    in_maps = []
    for i in core_ids:
        sl = slice(i * B_LOC, (i + 1) * B_LOC)
        in_maps.append(
            {
                "x_sh": x[sl],
                "a_sh": A[sl],
                "pp_sh": phys_prior[sl],
                "ones_bd": ones_bd,
                "w1r": w1r,
                "b1r": b1r,
                "w2r": w2r_dt,
                "cvec": cvec,
                "pwpb": pwpb,
            }
        )

    res = run_bass_kernel_spmd(nc, in_maps, core_ids)
    out = np.concatenate([res.results[i]["out"] for i in range(N_CORES)], axis=0)
    return out.astype(np.float32)


if __name__ == "__main__":
    # smoke test with random data
    rng = np.random.default_rng(0)
    inputs = dict(
        x=rng.standard_normal((B, F_DIM, C, M)).astype(np.float32),
        A=rng.random((B, M, C, C)).astype(np.float32),
        phys_prior=rng.standard_normal((B, C, M)).astype(np.float32),
        kappa=np.float32(0.1),
        alpha=np.float32(0.05),
        w1=rng.standard_normal((16, 1)).astype(np.float32),
        b1=np.zeros(16, np.float32),
        w2=(rng.standard_normal((1, 16)) * 0.25).astype(np.float32),
        b2=np.zeros(1, np.float32),
        pw=rng.standard_normal(OUT_CH).astype(np.float32),
        pb=np.zeros(OUT_CH, np.float32),
    )
    out = kernel(**inputs)
    print("out", out.shape, out.dtype)


# revision 30
# speedup vs baseline: 1.0495x; 1.0495x over previous
"""Trainium2 Bass kernel for nn_DiffusionLayer (gnn_message_passing).

Computation (full shapes):
  x (16,64,64,512), A (16,512,64,64), phys_prior (16,64,512) ->
  corr (16,32,64,512)

Sharding: pure data parallel over batch B=16 across 8 cores (B_LOC=2 each).
All reductions are local to a (b, m) tile; scalar params replicated.

Per-core layout convention: SBUF partition index p = b_loc*64 + c  (128
partitions), free dim = m (512).  This makes every elementwise stage a
[128, 512] tile and every DMA partition-major with >=256B runs.

Stages:
  1. s = mean_f x           : PE matmul with a block-diagonal ones lhsT
                              (K=(f2,c)=128, N=512), accumulated in PSUM.
  2. deg^T[(b,c), m]        : DVE reduce over innermost d of A tiles
                              [128=(b,c), (m_T, d)].
  3. As^T[(b,c), m]         : 2 small PE matmuls per m (per b): out[d,1] =
                              A[b,m](c,d)^T @ s[b,:,m], written directly at
                              psum partitions b*64..b*64+64 (PE quadrant
                              tiling), one psum bank holds all of As^T.
  4. tiny MLP r(b,c)        : DVE/ACT ops on [128, 16] tiles.
  5. combine                : s_new = s*(1-DT*k*deg) + DT*k*As + DT*alpha*pp
                              + DT*r    (DVE, [128,512] tiles)
  6. out[o] = s_new*pw[o]+pb[o] : DVE tensor_scalar per channel, DMA out.
"""

import os
import sys
import math
import numpy as np

sys.path.insert(0, "/opt/trn_rl_repo")

import concourse.bass as bass  # noqa: E402
from concourse import bacc  # noqa: E402
import concourse.tile as tile  # noqa: E402
from concourse import mybir  # noqa: E402
from concourse.bass_utils import run_bass_kernel_spmd  # noqa: E402

B, F_DIM, C, M = 16, 64, 64, 512
OUT_CH = 32
DT = 1.0
N_CORES = 8
B_LOC = B // N_CORES  # 2
F32 = mybir.dt.float32
M_T = 32  # m's per A tile

_CACHE = {}


def _build_bass():
    nc = bacc.Bacc()

    x_sh = nc.declare_dram_parameter("x_sh", [B_LOC, F_DIM, C, M], F32, isOutput=False)
    a_sh = nc.declare_dram_parameter("a_sh", [B_LOC, M, C, C], F32, isOutput=False)
    pp_sh = nc.declare_dram_parameter("pp_sh", [B_LOC, C, M], F32, isOutput=False)
    ones_bd = nc.declare_dram_parameter("ones_bd", [128, C], F32, isOutput=False)
    w1r = nc.declare_dram_parameter("w1r", [128, 16], F32, isOutput=False)
    b1r = nc.declare_dram_parameter("b1r", [128, 16], F32, isOutput=False)
    w2r = nc.declare_dram_parameter("w2r", [128, 16], F32, isOutput=False)
    cvec = nc.declare_dram_parameter("cvec", [128, 4], F32, isOutput=False)
    pwpb = nc.declare_dram_parameter("pwpb", [128, 2 * OUT_CH], F32, isOutput=False)
    out_sh = nc.declare_dram_parameter("out", [B_LOC, OUT_CH, C, M], F32, isOutput=True)

    AX = mybir.AxisListType
    OP = mybir.AluOpType
    ACTF = mybir.ActivationFunctionType

    with tile.TileContext(nc) as tc:
        with (
            tc.tile_pool(name="const", bufs=1) as cpool,
            tc.tile_pool(name="xp", bufs=3) as xpool,
            tc.tile_pool(name="ap", bufs=10) as apool,
            tc.tile_pool(name="sp", bufs=1) as spool,
            tc.tile_pool(name="tmp", bufs=1) as tpool,
            tc.tile_pool(name="dpk", bufs=4) as dpkpool,
            tc.tile_pool(name="small", bufs=1) as smpool,
            tc.tile_pool(name="op", bufs=2) as opool,
            tc.tile_pool(name="ps_s", bufs=1, space="PSUM") as ps_s_pool,
            tc.tile_pool(name="ps_as", bufs=3, space="PSUM") as ps_as_pool,
        ):
            # ---- constants ----
            ones_t = cpool.tile([128, C], F32)
            nc.sync.dma_start(ones_t[:], ones_bd[:])
            w1r_t = cpool.tile([128, 16], F32)
            nc.sync.dma_start(w1r_t[:], w1r[:])
            b1r_t = cpool.tile([128, 16], F32)
            nc.sync.dma_start(b1r_t[:], b1r[:])
            w2r_t = cpool.tile([128, 16], F32)
            nc.sync.dma_start(w2r_t[:], w2r[:])
            cvec_t = cpool.tile([128, 4], F32)
            nc.sync.dma_start(cvec_t[:], cvec[:])
            pwpb_t = cpool.tile([128, 2 * OUT_CH], F32)
            nc.sync.dma_start(pwpb_t[:], pwpb[:])
            pp_t = spool.tile([128, M], F32)
            nc.sync.dma_start(pp_t[:], pp_sh[:])

            # ---- stage 1: s = mean_f x  (PE blockdiag-ones matmul) ----
            # One DMA covers FPG f-pairs (2 MiB transfers -> all 16 SDMA
            # engines; 2 KiB descriptors).  Free layout (fp, m).
            s_ps = ps_s_pool.tile([128, M], F32)
            s_t = spool.tile([128, M], F32)
            s_bds = []
            NFP = F_DIM // 2  # f-pairs
            FPG = 8  # f-pairs per DMA
            for b in range(B_LOC):
                for fg in range(NFP // FPG):
                    xt = xpool.tile([128, FPG * M], F32)
                    # in: (fp, f2, c, m); out traversal (f2, c | fp, m)
                    xin = x_sh[b, 2 * fg * FPG : 2 * (fg + 1) * FPG].rearrange(
                        "(fp ftwo) c m -> ftwo c fp m", ftwo=2
                    )
                    nc.sync.dma_start(xt[:].rearrange("p (fp m) -> p fp m", m=M), xin)
                    for g in range(FPG):
                        fp = fg * FPG + g
                        nc.tensor.matmul(
                            s_ps[b * C : (b + 1) * C, :],
                            ones_t[:],
                            xt[:, g * M : (g + 1) * M],
                            start=(fp == 0),
                            stop=(fp == NFP - 1),
                        )
                # s and the blockdiag-s for this b become available as soon
                # as this b's x stream finishes -> b's As matmuls can start
                # draining A tiles while the other b's x is still streaming.
                bsl = slice(b * C, (b + 1) * C)
                nc.scalar.activation(
                    s_t[bsl, :], s_ps[bsl, :], ACTF.Copy, scale=1.0 / F_DIM
                )
                bb = spool.tile([128, M], F32, tag=f"sbd{b}", name=f"sbd{b}")
                nc.vector.memset(bb[:], 0.0)
                nc.vector.tensor_copy(bb[0:64, 0::2], s_t[bsl, 0::2])
                nc.vector.tensor_copy(bb[64:128, 1::2], s_t[bsl, 1::2])
                s_bds.append(bb)

            # ---- stage 4 (early): tiny MLP on r_in = mean_m s ----
            rsum = smpool.tile([128, 1], F32)
            nc.vector.tensor_reduce(rsum[:], s_t[:], axis=AX.X, op=OP.add)
            rin = smpool.tile([128, 1], F32)
            nc.vector.tensor_scalar_mul(rin[:], rsum[:], 1.0 / M)
            hp = smpool.tile([128, 16], F32)
            nc.vector.tensor_scalar(hp[:], w1r_t[:], rin[:], None, op0=OP.mult)
            nc.vector.tensor_add(hp[:], hp[:], b1r_t[:])
            hneg = smpool.tile([128, 16], F32)
            nc.vector.tensor_scalar_min(hneg[:], hp[:], 0.0)
            hexp = smpool.tile([128, 16], F32)
            nc.scalar.activation(hexp[:], hneg[:], ACTF.Exp)
            hrelu = smpool.tile([128, 16], F32)
            nc.vector.tensor_scalar_max(hrelu[:], hp[:], 0.0)
            helu = smpool.tile([128, 16], F32)
            nc.vector.tensor_add(helu[:], hexp[:], hrelu[:])
            # helu currently = elu + 1 ; fold the -1 into rdt via dot with w2r:
            # sum(w2r*(elu+1)) = sum(w2r*elu) + sum(w2r)  -> subtract sum(w2r)
            hw = smpool.tile([128, 16], F32)
            nc.vector.tensor_mul(hw[:], helu[:], w2r_t[:])
            rpre = smpool.tile([128, 1], F32)
            nc.vector.tensor_reduce(rpre[:], hw[:], axis=AX.X, op=OP.add)
            # rdt = rpre - sum(w2r) + DT*b2  (host folds both into cvec[:,3])
            rdt = smpool.tile([128, 1], F32)
            nc.vector.tensor_scalar(rdt[:], rpre[:], cvec_t[:, 3:4], None, op0=OP.add)

            # ---- stages 2+3+5+6, software-pipelined in m-quarters ----
            # Emit quarter q's combine+out AFTER quarter q+1's A-loop so the
            # DVE FIFO never blocks the A stream; out-DMAs ride the scalar
            # HWDGE ring so they cannot head-of-line-block A-DMAs (sync ring).
            deg_t = spool.tile([128, M], F32)
            snew = spool.tile([128, M], F32)
            MH = M_T // 2  # m-pairs per tile
            NQ = 4
            MBH = M // NQ  # m's per quarter
            OG = 8  # out channels per DMA

            as_tiles = {}

            def emit_a_quarter(q):
                as_tiles[q] = []
                for b in range(B_LOC):
                    aspb = ps_as_pool.tile(
                        [128, MBH], F32, tag=f"asps{b}", name=f"asps{b}_{q}"
                    )
                    as_tiles[q].append(aspb)
                for mt in range(q * (MBH // M_T), (q + 1) * (MBH // M_T)):
                    for b in range(B_LOC):
                        at = apool.tile([128, MH * C], F32, tag=f"at{b}")
                        ain = a_sh[b, mt * M_T : (mt + 1) * M_T].rearrange(
                            "(m1 m0) c d -> m0 c m1 d", m0=2
                        )
                        nc.sync.dma_start(
                            at[:].rearrange("p (m d) -> p m d", d=C), ain
                        )
                        dpk = dpkpool.tile([128, MH], F32, tag="dpk")
                        at3 = at[:].rearrange("p (mm d) -> p mm d", d=C)
                        nc.vector.tensor_reduce(dpk[:], at3, axis=AX.X, op=OP.add)
                        bsl = slice(b * C, (b + 1) * C)
                        nc.vector.tensor_copy(
                            deg_t[bsl, mt * M_T : (mt + 1) * M_T : 2], dpk[0:64, :]
                        )
                        nc.vector.tensor_copy(
                            deg_t[bsl, mt * M_T + 1 : (mt + 1) * M_T : 2],
                            dpk[64:128, :],
                        )
                        for j in range(MH // 2):
                            # [128,128] weight covers 4 m's (one LDW);
                            # rhs = 4 blockdiag-s cols; out rows (m1p, d),
                            # psum col == m - q*MBH
                            me4 = mt * M_T + 4 * j
                            mq = me4 - q * MBH
                            nc.tensor.matmul(
                                as_tiles[q][b][:, mq : mq + 4],
                                at[:, 2 * j * C : (2 * j + 2) * C],
                                s_bds[b][:, me4 : me4 + 4],
                                start=True,
                                stop=True,
                            )

            def emit_combine_out(q):
                as_ps_b = as_tiles.pop(q)
                hs = slice(q * MBH, (q + 1) * MBH)
                t2p = tpool.tile([128, MBH], F32, tag="t2p")
                nc.vector.tensor_scalar(
                    t2p[:], deg_t[:, hs], cvec_t[:, 0:1], 1.0, op0=OP.mult, op1=OP.add
                )
                t2 = tpool.tile([128, MBH], F32, tag="t2")
                nc.vector.tensor_mul(t2[:], t2p[:], s_t[:, hs])
                # t3 = DT*k*As: psum rows (m1-parity, d); valid half by
                # (m//2)%2: cols {4u,4u+1} -> rows 0:64, {4u+2,4u+3} -> 64:128
                t3 = tpool.tile([128, MBH], F32, tag="t3")
                kap = cvec_t[0:64, 1:2]
                for b in range(B_LOC):
                    bsl = slice(b * C, (b + 1) * C)
                    aps = as_ps_b[b]
                    t3v = t3[bsl, :].rearrange("p (u k) -> p u k", k=4)
                    apse = aps[0:64, :].rearrange("p (u k) -> p u k", k=4)
                    apso = aps[64:128, :].rearrange("p (u k) -> p u k", k=4)
                    nc.vector.tensor_scalar(
                        t3v[:, :, 0:2], apse[:, :, 0:2], kap, None, op0=OP.mult
                    )
                    nc.vector.tensor_scalar(
                        t3v[:, :, 2:4], apso[:, :, 2:4], kap, None, op0=OP.mult
                    )
                t4 = tpool.tile([128, MBH], F32, tag="t4")
                nc.vector.tensor_add(t4[:], t2[:], t3[:])
                t5 = tpool.tile([128, MBH], F32, tag="t5")
                nc.vector.tensor_scalar(
                    t5[:], pp_t[:, hs], cvec_t[:, 2:3], rdt[:], op0=OP.mult, op1=OP.add
                )
                nc.vector.tensor_add(snew[:, hs], t4[:], t5[:])
                for og in range(OUT_CH // OG):
                    ot = opool.tile([128, OG * MBH], F32, tag="ot")
                    for g in range(OG):
                        o = og * OG + g
                        if g % 2 == 0:
                            nc.vector.tensor_scalar(
                                ot[:, g * MBH : (g + 1) * MBH],
                                snew[:, hs],
                                pwpb_t[:, 2 * o : 2 * o + 1],
                                pwpb_t[:, 2 * o + 1 : 2 * o + 2],
                                op0=OP.mult,
                                op1=OP.add,
                            )
                        else:
                            nc.scalar.activation(
                                ot[:, g * MBH : (g + 1) * MBH],
                                snew[:, hs],
                                ACTF.Identity,
                                bias=pwpb_t[:, 2 * o + 1 : 2 * o + 2],
                                scale=pwpb_t[:, 2 * o : 2 * o + 1],
                            )
                    for b in range(B_LOC):
                        odst = out_sh[
                            b, og * OG : (og + 1) * OG, :, q * MBH : (q + 1) * MBH
                        ].rearrange("o c m -> c o m")
                        osrc = ot[b * C : (b + 1) * C, :].rearrange(
                            "p (o m) -> p o m", m=MBH
                        )
                        nc.scalar.dma_start(odst, osrc)

            for q in range(NQ):
                emit_a_quarter(q)
                if q >= 1:
                    emit_combine_out(q - 1)
            emit_combine_out(NQ - 1)

    nc.compile()
    return nc


def _get_bass():
    if "nc" not in _CACHE:
        _CACHE["nc"] = _build_bass()
    return _CACHE["nc"]


def _host_consts(kappa, alpha, w1, b1, w2, b2, pw, pb):
    kappa = float(np.asarray(kappa))
    alpha = float(np.asarray(alpha))
    w1 = np.asarray(w1, np.float32).reshape(16, 1)
    b1 = np.asarray(b1, np.float32).reshape(16)
    w2 = np.asarray(w2, np.float32).reshape(1, 16)
    b2 = np.asarray(b2, np.float32).reshape(1)
    pw = np.asarray(pw, np.float32).reshape(OUT_CH)
    pb = np.asarray(pb, np.float32).reshape(OUT_CH)

    kDT = DT * float(np.log1p(np.exp(kappa)))  # DT * softplus(kappa)

    ones_bd = np.zeros((128, C), np.float32)
    for f in range(2):
        for c in range(C):
            ones_bd[f * C + c, c] = 1.0

    w1r = np.tile(w1.T.astype(np.float32), (128, 1))  # [128,16]
    b1r = np.tile(b1[None, :], (128, 1)).astype(np.float32)
    w2r_dt = np.tile((DT * w2).astype(np.float32), (128, 1))  # [128,16]

    cvec = np.zeros((128, 4), np.float32)
    cvec[:, 0] = -kDT
    cvec[:, 1] = kDT
    cvec[:, 2] = DT * alpha
    # rdt = rpre + cvec3 where rpre = sum(w2r_dt * (elu+1));
    # true DT*r = sum(w2r_dt*elu) + DT*b2  ->  cvec3 = DT*b2 - sum(w2r_dt row)
    cvec[:, 3] = DT * b2[0] - float(w2r_dt[0].sum())

    pwpb = np.zeros((128, 2 * OUT_CH), np.float32)
    pwpb[:, 0::2] = pw[None, :]
    pwpb[:, 1::2] = pb[None, :]
    return ones_bd, w1r, b1r, w2r_dt, cvec, pwpb


def kernel(x, A, phys_prior, kappa, alpha, w1, b1, w2, b2, pw, pb):
    x = np.ascontiguousarray(np.asarray(x, np.float32))
    A = np.ascontiguousarray(np.asarray(A, np.float32))
    phys_prior = np.ascontiguousarray(np.asarray(phys_prior, np.float32))
    ones_bd, w1r, b1r, w2r_dt, cvec, pwpb = _host_consts(
        kappa, alpha, w1, b1, w2, b2, pw, pb
    )

    nc = _get_bass()
    core_ids = list(range(N_CORES))
    in_maps = []
    for i in core_ids:
        sl = slice(i * B_LOC, (i + 1) * B_LOC)
        in_maps.append(
            {
                "x_sh": x[sl],
                "a_sh": A[sl],
                "pp_sh": phys_prior[sl],
                "ones_bd": ones_bd,
                "w1r": w1r,
                "b1r": b1r,
                "w2r": w2r_dt,
                "cvec": cvec,
                "pwpb": pwpb,
            }
        )

    res = run_bass_kernel_spmd(nc, in_maps, core_ids)
    out = np.concatenate([res.results[i]["out"] for i in range(N_CORES)], axis=0)
    return out.astype(np.float32)


if __name__ == "__main__":
    # smoke test with random data
    rng = np.random.default_rng(0)
    inputs = dict(
        x=rng.standard_normal((B, F_DIM, C, M)).astype(np.float32),
        A=rng.random((B, M, C, C)).astype(np.float32),
        phys_prior=rng.standard_normal((B, C, M)).astype(np.float32),
        kappa=np.float32(0.1),
        alpha=np.float32(0.05),
        w1=rng.standard_normal((16, 1)).astype(np.float32),
        b1=np.zeros(16, np.float32),
        w2=(rng.standard_normal((1, 16)) * 0.25).astype(np.float32),
        b2=np.zeros(1, np.float32),
        pw=rng.standard_normal(OUT_CH).astype(np.float32),
        pb=np.zeros(OUT_CH, np.float32),
    )
    out = kernel(**inputs)
    print("out", out.shape, out.dtype)
